# revision 34
# baseline (speedup 1.0000x reference)
"""Hawk (RG-LRU) Trainium2 kernel.

Full-input contract: kernel(**inputs) takes the unsharded inputs from
setup_inputs() and returns the full [N, T, DIM] output.

Sharding (8 cores): core = 2n + c -> batch n in 0..3, channel-half c in {0,1}
(768 of the 1536 hidden channels). No cross-core communication: each core
computes the FULL conv'd xh (needed as the contraction input of the gates
matmul) but only its channel-half of the gates / recurrence / output
projection. The two per-n partial outputs are summed on the host.

Per-core pipeline, channels-on-partitions / time-on-free, bf16 matmuls:
  A) xt resident in SBUF (bf16, streamed in 512-col chunks), mm1 with
     w1 streamed once (no re-loads), xh tiles in bf16, causal depthwise
     conv as 4 shifted multiply-adds (DVE/Pool split), gate tiles via
     direct Gelu on ACT.
  B) per channel-tile p: forget/input matmuls (K=1536) into PSUM,
     sigmoid/exp via tanh identities (single ACT table + sqrt),
     alpha = exp(pc*sig(f)), bxh = 0.5*beta*xh, xi = (1+tanh_i)*bxh,
     recurrence via chunked tensor_tensor_scan with carry, gh = ge*h.
  C) out = W3 @ gh (K=768), DMA straight from PSUM.
"""

import numpy as np

import concourse.bacc as bacc
import concourse.mybir as mybir
import concourse.tile as tile
from concourse.bass_utils import run_bass_kernel_spmd

f32 = mybir.dt.float32
f32r = mybir.dt.float32r
bf16 = mybir.dt.bfloat16
AF = mybir.ActivationFunctionType
ALU = mybir.AluOpType


def build_nc(D, HID, HS, T, num_cores=8):
    """Build + compile the per-core SPMD program. All cores run this same
    program on different data."""
    KD, KH, NP = D // 128, HID // 128, HS // 128
    MH = KH + NP          # mm1 output tiles: KH xh tiles then NP gate tiles
    MD = D // 128
    NCQ = T // 512        # 512-col chunks

    nc = bacc.Bacc("TRN2", target_bir_lowering=False, debug=False,
                   num_devices=num_cores)

    xt_d = nc.dram_tensor("xt", [128, KD, T], bf16, kind="ExternalInput")
    w1_d = nc.dram_tensor("w1", [MH, 128, KD * 128], bf16, kind="ExternalInput")
    cw_d = nc.dram_tensor("cw", [128, KH, 4], f32, kind="ExternalInput")
    cb_d = nc.dram_tensor("cb", [128, KH], f32, kind="ExternalInput")
    w2_d = nc.dram_tensor("w2", [2 * NP, 128, KH * 128], bf16, kind="ExternalInput")
    gbh_d = nc.dram_tensor("gbh", [128, 2 * NP], f32, kind="ExternalInput")
    pch_d = nc.dram_tensor("pch", [128, NP], f32, kind="ExternalInput")
    w3_d = nc.dram_tensor("w3", [NP, 128, D], f32r, kind="ExternalInput")
    out_d = nc.dram_tensor("o", [D, T], bf16, kind="ExternalOutput")

    POOL_CONV = ()   # TensorScalarPtr is DVE-only on real HW (no Pool conv)

    with tile.TileContext(nc) as tc:
        # pool alloc order is LIFO-release: consts/ge/w3 live to the end,
        # xh dies after phase B, xt after phase A
        consts = tc.alloc_tile_pool(name="consts", bufs=1)
        gep = tc.alloc_tile_pool(name="ge", bufs=NP)
        w3p = tc.alloc_tile_pool(name="w3", bufs=NP)
        xhp = tc.alloc_tile_pool(name="xh", bufs=KH)
        # w2 sits below xt on the alloc stack so its tiles don't alias xt's
        # space -> phase B's first weight DMAs don't wait for xt to die
        w2p = tc.alloc_tile_pool(name="w2", bufs=4)
        xtp = tc.alloc_tile_pool(name="xt", bufs=1)

        # x resident for all of phase A, streamed chunk-major so the first
        # matmul only waits on the first 512-col chunk. DMA issue order
        # matters: the single DMA pipe drains in order, so get the first
        # chunk + first w1 tiles out before the bulk of x.
        xt = xtp.tile([128, KD, T], bf16, tag="xt")
        nc.sync.dma_start(xt[:, :, 0:256], xt_d[:, :, 0:256])
        nc.sync.dma_start(xt[:, :, 256:512], xt_d[:, :, 256:512])

        w1p = tc.alloc_tile_pool(name="w1", bufs=3)
        w1head = []
        for m in range(3):
            w1m = w1p.tile([128, KD, 128], bf16, tag="w1")
            nc.sync.dma_start(
                w1m[:], w1_d[m].rearrange("p (k f) -> p k f", k=KD))
            w1head.append(w1m)

        for cq in range(1, NCQ):
            s = cq * 512
            nc.sync.dma_start(xt[:, :, s:s + 512], xt_d[:, :, s:s + 512])

        cw = consts.tile([128, KH, 4], f32, tag="cw")
        nc.sync.dma_start(cw[:], cw_d[:])
        cb = consts.tile([128, KH], f32, tag="cb")
        nc.sync.dma_start(cb[:], cb_d[:])
        gbh = consts.tile([128, 2 * NP], f32, tag="gbh")
        nc.sync.dma_start(gbh[:], gbh_d[:])
        pch = consts.tile([128, NP], f32, tag="pch")
        nc.sync.dma_start(pch[:], pch_d[:])
        qrt = consts.tile([128, 1], f32, tag="qrt")
        nc.gpsimd.memset(qrt[:], 0.25)

        # gelu(gate) tiles, resident through phase C (become gh in phase B)
        ge = [gep.tile([128, T], f32r, tag="ge", name=f"ge{g}")
              for g in range(NP)]

        xh = [xhp.tile([128, T + 4], bf16, tag="xh", name=f"xh{m}")
              for m in range(KH)]
        for m in range(KH):
            nc.gpsimd.memset(xh[m][:, 0:4].bitcast(f32), 0.0)

        # ---------------- Phase A: mm1 + conv + gelu(gate) ----------------
        with (
            tc.tile_pool(name="accv", bufs=2) as accvp,
            tc.tile_pool(name="accp", bufs=2) as accpp,
            tc.tile_pool(name="psA", bufs=4, space="PSUM") as psa,
        ):
            for m in range(MH):
                if m < 3:
                    w1m = w1head[m]
                else:
                    w1m = w1p.tile([128, KD, 128], bf16, tag="w1")
                    nc.sync.dma_start(
                        w1m[:], w1_d[m].rearrange("p (k f) -> p k f", k=KD))
                for cq in range(NCQ):
                    s = cq * 512
                    ps = psa.tile([128, 512], f32)
                    for k in range(KD):
                        nc.tensor.matmul(
                            ps[:],
                            w1m[:, k, :],
                            xt[:, k, s:s + 512],
                            start=(k == 0),
                            stop=(k == KD - 1),
                        )
                    if m < KH:
                        nc.scalar.copy(xh[m][:, 4 + s:4 + s + 512], ps[:])
                    else:
                        nc.scalar.activation(ge[m - KH][:, s:s + 512], ps[:],
                                             AF.Gelu)
                if m < KH:
                    # causal depthwise conv, 4 shifted multiply-adds.
                    # taps 0-2 read raw (not yet convolved) columns via the
                    # 4-zero left pad; the final tap overwrites in place.
                    eng = nc.gpsimd if m in POOL_CONV else nc.vector
                    accp_ = accpp if m in POOL_CONV else accvp
                    acc = accp_.tile([128, T], bf16, tag="acc")
                    eng.tensor_scalar(
                        acc[:], xh[m][:, 1:1 + T],
                        cw[:, m, 0:1], cb[:, m:m + 1],
                        ALU.mult, ALU.add)
                    for tap in (1, 2):
                        eng.scalar_tensor_tensor(
                            acc[:], xh[m][:, 1 + tap:1 + tap + T],
                            cw[:, m, tap:tap + 1],
                            acc[:], ALU.mult, ALU.add)
                    eng.scalar_tensor_tensor(
                        xh[m][:, 4:4 + T], xh[m][:, 4:4 + T],
                        cw[:, m, 3:4], acc[:], ALU.mult, ALU.add)
        w1p.release()
        xtp.release()

        # ---------------- Phase B: mm2 + gates + scan + gh ----------------
        with (
            tc.tile_pool(name="alp", bufs=2) as alp,
            tc.tile_pool(name="bsc", bufs=2) as bscp,
            tc.tile_pool(name="tip", bufs=2) as tip,
            tc.tile_pool(name="xip", bufs=2) as xip,
            tc.tile_pool(name="psB", bufs=2, space="PSUM") as psb,
        ):
            # first gate-weight pair ahead of the w3 prefetch: the DMA pipe
            # drains in order and phase B's first matmul needs w2[0]
            w2head = []
            for g in (0, NP):
                w2g = w2p.tile([128, KH, 128], bf16, tag="w2")
                nc.sync.dma_start(
                    w2g[:], w2_d[g].rearrange("p (k f) -> p k f", k=KH))
                w2head.append(w2g)
            # w3 prefetch (used in phase C; DMA overlaps phase B compute)
            w3 = []
            for k in range(NP):
                w3k = w3p.tile([128, D], f32r, tag="w3", name=f"w3_{k}")
                nc.sync.dma_start(w3k[:], w3_d[k])
                w3.append(w3k)

            for p in range(NP):
                # forget gate matmul
                if p == 0:
                    w2f = w2head[0]
                else:
                    w2f = w2p.tile([128, KH, 128], bf16, tag="w2")
                    nc.sync.dma_start(
                        w2f[:], w2_d[p].rearrange("p (k f) -> p k f", k=KH))
                psf = psb.tile([128, T], f32, tag="psB")
                for k in range(KH):
                    for h in range(NCQ):
                        hs = h * 512
                        nc.tensor.matmul(
                            psf[:, hs:hs + 512],
                            w2f[:, k, :],
                            xh[k][:, 4 + hs:4 + hs + 512],
                            start=(k == 0),
                            stop=(k == KH - 1),
                        )
                # alpha = exp(pc * sigmoid(f)) via tanh identity:
                #   sigmoid(f) = 0.5 + 0.5*tanh(0.5 f);  pch = 0.5*pc
                alpha = alp.tile([128, T], f32, tag="alp")
                nc.scalar.activation(alpha[:], psf[:], AF.Tanh,
                                     bias=gbh[:, p:p + 1], scale=0.5)
                nc.scalar.activation(alpha[:], alpha[:], AF.Exp,
                                     bias=pch[:, p:p + 1],
                                     scale=pch[:, p:p + 1])
                # bxh = 0.5*beta*xh, with 0.5*beta = sqrt(0.25 - 0.25 a^2)
                bsc = bscp.tile([128, T], f32, tag="bsc")
                nc.vector.tensor_mul(bsc[:], alpha[:], alpha[:])
                nc.scalar.activation(bsc[:], bsc[:], AF.Sqrt,
                                     bias=qrt[:, 0:1], scale=-0.25)
                nc.vector.tensor_mul(bsc[:], bsc[:], xh[p][:, 4:4 + T])
                # input gate matmul
                if p == 0:
                    w2i = w2head[1]
                else:
                    w2i = w2p.tile([128, KH, 128], bf16, tag="w2")
                    nc.sync.dma_start(
                        w2i[:], w2_d[NP + p].rearrange("p (k f) -> p k f", k=KH))
                psi = psb.tile([128, T], f32, tag="psB")
                for k in range(KH):
                    for h in range(NCQ):
                        hs = h * 512
                        nc.tensor.matmul(
                            psi[:, hs:hs + 512],
                            w2i[:, k, :],
                            xh[k][:, 4 + hs:4 + hs + 512],
                            start=(k == 0),
                            stop=(k == KH - 1),
                        )
                # chunked epilogue: sigmoid_i via tanh, xi = (1+t)*bxh,
                # scan with carry, gh = ge*h  (short tail after last matmul)
                ti = tip.tile([128, T], bf16, tag="tip")
                xi = xip.tile([128, T], f32, tag="xip")
                for cq in range(NCQ):
                    s = cq * 512
                    sl = slice(s, s + 512)
                    nc.scalar.activation(ti[:, sl], psi[:, sl], AF.Tanh,
                                         bias=gbh[:, NP + p:NP + p + 1],
                                         scale=0.5)
                    nc.vector.scalar_tensor_tensor(
                        xi[:, sl], ti[:, sl], 1.0, bsc[:, sl],
                        ALU.add, ALU.mult)
                    nc.vector.tensor_tensor_scan(
                        xi[:, sl], alpha[:, sl], xi[:, sl],
                        0.0 if cq == 0 else xi[:, s - 1:s],
                        ALU.mult, ALU.add)
                    nc.gpsimd.tensor_mul(ge[p][:, sl], ge[p][:, sl],
                                         xi[:, sl])
        w2p.release()
        xhp.release()

        # ---------------- Phase C: mm3, DMA straight from PSUM ----------------
        with (
            tc.tile_pool(name="outp", bufs=2) as outp,
            tc.tile_pool(name="psC", bufs=2, space="PSUM") as psc,
        ):
            for m in range(MD):
                ps = psc.tile([128, T], f32)
                for k in range(NP):
                    for h in range(NCQ):
                        hs = h * 512
                        nc.tensor.matmul(
                            ps[:, hs:hs + 512],
                            w3[k][:, m * 128:(m + 1) * 128],
                            ge[k][:, hs:hs + 512],
                            start=(k == 0),
                            stop=(k == NP - 1),
                        )
                ot = outp.tile([128, T], bf16, tag="outp")
                for cq in range(NCQ):
                    s = cq * 512
                    sl = slice(s, s + 512)
                    if cq % 2 == 0:
                        nc.scalar.copy(ot[:, sl], ps[:, sl])
                    else:
                        nc.vector.tensor_copy(ot[:, sl], ps[:, sl])
                    nc.sync.dma_start(out_d[m * 128:(m + 1) * 128, sl],
                                      ot[:, sl])
        w3p.release()
        gep.release()
        consts.release()

    nc.compile()
    return nc


def make_in_maps(x, input_w, conv_w, conv_b, gates_w, gates_b, forget_base,
                 output_w, D, HID, HS, T, num_cores):
    KD, KH, NP = D // 128, HID // 128, HS // 128
    N = x.shape[0]
    np_bf16 = mybir.dt.np(bf16)
    in_maps = []
    for core in range(num_cores):
        n, c = core // (num_cores // N), core % (num_cores // N)
        own = np.arange(c * HS, (c + 1) * HS)
        other = np.concatenate(
            [np.arange(0, c * HS), np.arange((c + 1) * HS, HID)])
        perm = np.concatenate([own, other])

        xt = np.ascontiguousarray(
            x[n].T.reshape(KD, 128, T).transpose(1, 0, 2)).astype(np_bf16)

        w1sel = np.concatenate([input_w[HID:2 * HID][perm], input_w[own]], 0)
        w1T = w1sel.T  # [D, HID+HS]
        MH = KH + NP
        w1 = np.stack([
            np.ascontiguousarray(
                w1T[:, m * 128:(m + 1) * 128].reshape(KD, 128, 128)
                .transpose(1, 0, 2)).reshape(128, KD * 128)
            for m in range(MH)
        ]).astype(np_bf16)

        cw = np.ascontiguousarray(
            conv_w[perm, 0, :].reshape(KH, 128, 4).transpose(1, 0, 2)
        ).astype(np.float32)
        cb = np.ascontiguousarray(
            conv_b[perm].reshape(KH, 128).T).astype(np.float32)

        w2sel = np.concatenate([gates_w[own], gates_w[HID + own]], 0)
        w2T = w2sel.T[perm]  # [HID(perm order), 2*HS]
        w2 = np.stack([
            np.ascontiguousarray(
                w2T[:, g * 128:(g + 1) * 128].reshape(KH, 128, 128)
                .transpose(1, 0, 2)).reshape(128, KH * 128)
            for g in range(2 * NP)
        ]).astype(np_bf16)

        gbsel = 0.5 * np.concatenate([gates_b[own], gates_b[HID + own]])
        gbt = np.ascontiguousarray(gbsel.reshape(2 * NP, 128).T).astype(np.float32)

        pcv = (-4.0 * np.log1p(np.exp(forget_base.astype(np.float64))))
        pct = np.ascontiguousarray(
            pcv[own].reshape(NP, 128).T).astype(np.float32)

        w3 = np.ascontiguousarray(
            (output_w[:, own].T).reshape(NP, 128, D)).astype(np.float32)

        in_maps.append({
            "xt": xt, "w1": w1, "cw": cw, "cb": cb, "w2": w2,
            "gbh": gbt, "pch": pct, "w3": w3,
        })
    return in_maps


_CACHE = {}
TRACE = False
LAST_RES = None


def _get_nc(D, HID, HS, T, num_cores):
    key = (D, HID, HS, T, num_cores)
    if key not in _CACHE:
        _CACHE[key] = build_nc(D, HID, HS, T, num_cores)
    return _CACHE[key]


def run_hawk(x, input_w, conv_w, conv_b, gates_w, gates_b, forget_base,
             output_w, num_cores=8):
    N, T, D = x.shape
    HID = input_w.shape[0] // 2
    HS = HID // (num_cores // N)
    nc = _get_nc(D, HID, HS, T, num_cores)
    in_maps = make_in_maps(x, input_w, conv_w, conv_b, gates_w, gates_b,
                           forget_base, output_w, D, HID, HS, T, num_cores)
    global LAST_RES
    res = run_bass_kernel_spmd(nc, in_maps, core_ids=list(range(num_cores)),
                               trace=TRACE)
    LAST_RES = res
    shards_per_n = num_cores // N
    out = np.stack([
        sum(res.results[n * shards_per_n + c]["o"].astype(np.float32)
            for c in range(shards_per_n)).T
        for n in range(N)
    ])
    return np.ascontiguousarray(out.astype(np.float32))


def kernel(x, input_w, conv_w, conv_b, gates_w, gates_b, forget_base,
           output_w):
    return run_hawk(
        np.asarray(x, dtype=np.float32),
        np.asarray(input_w, dtype=np.float32),
        np.asarray(conv_w, dtype=np.float32),
        np.asarray(conv_b, dtype=np.float32),
        np.asarray(gates_w, dtype=np.float32),
        np.asarray(gates_b, dtype=np.float32),
        np.asarray(forget_base, dtype=np.float32),
        np.asarray(output_w, dtype=np.float32),
    )


# revision 42
# speedup vs baseline: 1.0238x; 1.0238x over previous
"""Hawk (RG-LRU) Trainium2 kernel.

Full-input contract: kernel(**inputs) takes the unsharded inputs from
setup_inputs() and returns the full [N, T, DIM] output.

Sharding (8 cores): core = 2n + c -> batch n in 0..3, channel-half c in {0,1}
(768 of the 1536 hidden channels). No cross-core communication: each core
computes the FULL conv'd xh (needed as the contraction input of the gates
matmul) but only its channel-half of the gates / recurrence / output
projection. The two per-n partial outputs are summed on the host.

Per-core pipeline, channels-on-partitions / time-on-free, bf16 matmuls:
  A) xt resident in SBUF (bf16, streamed in 512-col chunks), mm1 with
     w1 streamed once (no re-loads), xh tiles in bf16, causal depthwise
     conv as 4 shifted multiply-adds (DVE/Pool split), gate tiles via
     direct Gelu on ACT.
  B) per channel-tile p: forget/input matmuls (K=1536) into PSUM,
     sigmoid/exp via tanh identities (single ACT table + sqrt),
     alpha = exp(pc*sig(f)), bxh = 0.5*beta*xh, xi = (1+tanh_i)*bxh,
     recurrence via chunked tensor_tensor_scan with carry, gh = ge*h.
  C) out = W3 @ gh (K=768), DMA straight from PSUM.
"""

import numpy as np

import concourse.bacc as bacc
import concourse.mybir as mybir
import concourse.tile as tile
from concourse.bass_utils import run_bass_kernel_spmd

f32 = mybir.dt.float32
f32r = mybir.dt.float32r
bf16 = mybir.dt.bfloat16
AF = mybir.ActivationFunctionType
ALU = mybir.AluOpType


def build_nc(D, HID, HS, T, num_cores=8):
    """Build + compile the per-core SPMD program. All cores run this same
    program on different data."""
    KD, KH, NP = D // 128, HID // 128, HS // 128
    MH = KH + NP          # mm1 output tiles: KH xh tiles then NP gate tiles
    MD = D // 128
    NCQ = T // 512        # 512-col chunks

    nc = bacc.Bacc("TRN2", target_bir_lowering=False, debug=False,
                   num_devices=num_cores)

    xt_d = nc.dram_tensor("xt", [128, KD, T], bf16, kind="ExternalInput")
    w1_d = nc.dram_tensor("w1", [MH, 128, KD * 128], bf16, kind="ExternalInput")
    cw_d = nc.dram_tensor("cw", [128, KH, 4], f32, kind="ExternalInput")
    cb_d = nc.dram_tensor("cb", [128, KH], f32, kind="ExternalInput")
    w2_d = nc.dram_tensor("w2", [2 * NP, 128, KH * 128], bf16, kind="ExternalInput")
    gbh_d = nc.dram_tensor("gbh", [128, 2 * NP], f32, kind="ExternalInput")
    pch_d = nc.dram_tensor("pch", [128, NP], f32, kind="ExternalInput")
    w3_d = nc.dram_tensor("w3", [NP, 128, D], f32r, kind="ExternalInput")
    out_d = nc.dram_tensor("o", [D, T], bf16, kind="ExternalOutput")

    POOL_CONV = ()   # TensorScalarPtr is DVE-only on real HW (no Pool conv)

    with tile.TileContext(nc) as tc:
        # pool alloc order is LIFO-release: consts/ge/w3 live to the end,
        # xh dies after phase B, xt after phase A
        consts = tc.alloc_tile_pool(name="consts", bufs=1)
        gep = tc.alloc_tile_pool(name="ge", bufs=NP)
        w3p = tc.alloc_tile_pool(name="w3", bufs=NP)
        xhp = tc.alloc_tile_pool(name="xh", bufs=KH)
        # w2 sits below xt on the alloc stack so its tiles don't alias xt's
        # space -> phase B's first weight DMAs don't wait for xt to die
        w2p = tc.alloc_tile_pool(name="w2", bufs=4)
        xtp = tc.alloc_tile_pool(name="xt", bufs=1)

        # x resident for all of phase A, streamed chunk-major so the first
        # matmul only waits on the first 512-col chunk. DMA issue order
        # matters: the single DMA pipe drains in order, so get the first
        # chunk + first w1 tiles out before the bulk of x.
        xt = xtp.tile([128, KD, T], bf16, tag="xt")
        w1p = tc.alloc_tile_pool(name="w1", bufs=3)
        w1head = []

        nc.sync.dma_start(xt[:, :, 0:256], xt_d[:, :, 0:256])
        w1m0 = w1p.tile([128, KD, 128], bf16, tag="w1")
        nc.sync.dma_start(w1m0[:], w1_d[0].rearrange("p (k f) -> p k f", k=KD))
        w1head.append(w1m0)
        nc.sync.dma_start(xt[:, :, 256:512], xt_d[:, :, 256:512])
        for m in range(1, 3):
            w1m = w1p.tile([128, KD, 128], bf16, tag="w1")
            nc.sync.dma_start(
                w1m[:], w1_d[m].rearrange("p (k f) -> p k f", k=KD))
            w1head.append(w1m)

        for cq in range(1, NCQ):
            s = cq * 512
            nc.sync.dma_start(xt[:, :, s:s + 512], xt_d[:, :, s:s + 512])

        cw = consts.tile([128, KH, 4], f32, tag="cw")
        nc.sync.dma_start(cw[:], cw_d[:])
        cb = consts.tile([128, KH], f32, tag="cb")
        nc.sync.dma_start(cb[:], cb_d[:])
        gbh = consts.tile([128, 2 * NP], f32, tag="gbh")
        nc.sync.dma_start(gbh[:], gbh_d[:])
        pch = consts.tile([128, NP], f32, tag="pch")
        nc.sync.dma_start(pch[:], pch_d[:])
        qrt = consts.tile([128, 1], f32, tag="qrt")
        nc.gpsimd.memset(qrt[:], 0.25)

        # gelu(gate) tiles, resident through phase C (become gh in phase B)
        ge = [gep.tile([128, T], f32r, tag="ge", name=f"ge{g}")
              for g in range(NP)]

        xh = [xhp.tile([128, T + 4], bf16, tag="xh", name=f"xh{m}")
              for m in range(KH)]
        for m in range(KH):
            nc.gpsimd.memset(xh[m][:, 0:4].bitcast(f32), 0.0)

        # ---------------- Phase A: mm1 + conv + gelu(gate) ----------------
        with (
            tc.tile_pool(name="accv", bufs=2) as accvp,
            tc.tile_pool(name="accp", bufs=2) as accpp,
            tc.tile_pool(name="psA", bufs=4, space="PSUM") as psa,
        ):
            for m in range(MH):
                if m < 3:
                    w1m = w1head[m]
                else:
                    w1m = w1p.tile([128, KD, 128], bf16, tag="w1")
                    nc.sync.dma_start(
                        w1m[:], w1_d[m].rearrange("p (k f) -> p k f", k=KD))
                # m == 0 runs its first 512 cols as 2x256 so the very first
                # matmul only waits on the leading 256-col x DMA
                if m == 0:
                    chunks = [(0, 256), (256, 256)] + [
                        (s, 512) for s in range(512, T, 512)]
                else:
                    chunks = [(s, 512) for s in range(0, T, 512)]
                for s, CA in chunks:
                    ps = psa.tile([128, 512], f32)
                    for k in range(KD):
                        nc.tensor.matmul(
                            ps[:, 0:CA],
                            w1m[:, k, :],
                            xt[:, k, s:s + CA],
                            start=(k == 0),
                            stop=(k == KD - 1),
                        )
                    if m < KH:
                        nc.scalar.copy(xh[m][:, 4 + s:4 + s + CA],
                                       ps[:, 0:CA])
                    else:
                        nc.scalar.activation(ge[m - KH][:, s:s + CA],
                                             ps[:, 0:CA], AF.Gelu)
                if m < KH:
                    # causal depthwise conv, 4 shifted multiply-adds.
                    # taps 0-2 read raw (not yet convolved) columns via the
                    # 4-zero left pad; the final tap overwrites in place.
                    eng = nc.gpsimd if m in POOL_CONV else nc.vector
                    accp_ = accpp if m in POOL_CONV else accvp
                    acc = accp_.tile([128, T], bf16, tag="acc")
                    eng.tensor_scalar(
                        acc[:], xh[m][:, 1:1 + T],
                        cw[:, m, 0:1], cb[:, m:m + 1],
                        ALU.mult, ALU.add)
                    for tap in (1, 2):
                        eng.scalar_tensor_tensor(
                            acc[:], xh[m][:, 1 + tap:1 + tap + T],
                            cw[:, m, tap:tap + 1],
                            acc[:], ALU.mult, ALU.add)
                    eng.scalar_tensor_tensor(
                        xh[m][:, 4:4 + T], xh[m][:, 4:4 + T],
                        cw[:, m, 3:4], acc[:], ALU.mult, ALU.add)
        w1p.release()
        xtp.release()

        # ---------------- Phase B: mm2 + gates + scan + gh ----------------
        # psBi outlives phase B: if it were released with the other B pools,
        # phase C's PSUM tiles would alias it and wait on its last reader
        # (the final input-gate tanh) before C's first matmul could start.
        psbi = tc.alloc_tile_pool(name="psBi", bufs=1, space="PSUM")
        with (
            tc.tile_pool(name="alp", bufs=2) as alp,
            tc.tile_pool(name="bsc", bufs=2) as bscp,
            tc.tile_pool(name="tip", bufs=2) as tip,
            tc.tile_pool(name="xip", bufs=2) as xip,
            tc.tile_pool(name="psBf", bufs=1, space="PSUM") as psbf,
        ):
            # first gate-weight pair ahead of the w3 prefetch: the DMA pipe
            # drains in order and phase B's first matmul needs w2[0]
            w2head = []
            for g in (0, NP):
                w2g = w2p.tile([128, KH, 128], bf16, tag="w2")
                nc.sync.dma_start(
                    w2g[:], w2_d[g].rearrange("p (k f) -> p k f", k=KH))
                w2head.append(w2g)
            # w3 prefetch (used in phase C; DMA overlaps phase B compute)
            w3 = []
            for k in range(NP):
                w3k = w3p.tile([128, D], f32r, tag="w3", name=f"w3_{k}")
                nc.sync.dma_start(w3k[:], w3_d[k])
                w3.append(w3k)

            for p in range(NP):
                # forget gate matmul
                if p == 0:
                    w2f = w2head[0]
                else:
                    w2f = w2p.tile([128, KH, 128], bf16, tag="w2")
                    nc.sync.dma_start(
                        w2f[:], w2_d[p].rearrange("p (k f) -> p k f", k=KH))
                psf = psbf.tile([128, T], f32, tag="psBf")
                for k in range(KH):
                    for h in range(NCQ):
                        hs = h * 512
                        nc.tensor.matmul(
                            psf[:, hs:hs + 512],
                            w2f[:, k, :],
                            xh[k][:, 4 + hs:4 + hs + 512],
                            start=(k == 0),
                            stop=(k == KH - 1),
                        )
                # alpha = exp(pc * sigmoid(f)) via tanh identity:
                #   sigmoid(f) = 0.5 + 0.5*tanh(0.5 f);  pch = 0.5*pc
                alpha = alp.tile([128, T], f32, tag="alp")
                nc.scalar.activation(alpha[:], psf[:], AF.Tanh,
                                     bias=gbh[:, p:p + 1], scale=0.5)
                nc.scalar.activation(alpha[:], alpha[:], AF.Exp,
                                     bias=pch[:, p:p + 1],
                                     scale=pch[:, p:p + 1])
                # bxh = 0.5*beta*xh, with 0.5*beta = sqrt(0.25 - 0.25 a^2)
                bsc = bscp.tile([128, T], f32, tag="bsc")
                nc.vector.tensor_mul(bsc[:], alpha[:], alpha[:])
                nc.scalar.activation(bsc[:], bsc[:], AF.Sqrt,
                                     bias=qrt[:, 0:1], scale=-0.25)
                nc.vector.tensor_mul(bsc[:], bsc[:], xh[p][:, 4:4 + T])
                # input gate matmul
                if p == 0:
                    w2i = w2head[1]
                else:
                    w2i = w2p.tile([128, KH, 128], bf16, tag="w2")
                    nc.sync.dma_start(
                        w2i[:], w2_d[NP + p].rearrange("p (k f) -> p k f", k=KH))
                psi = psbi.tile([128, T], f32, tag="psBi")
                for k in range(KH):
                    for h in range(NCQ):
                        hs = h * 512
                        nc.tensor.matmul(
                            psi[:, hs:hs + 512],
                            w2i[:, k, :],
                            xh[k][:, 4 + hs:4 + hs + 512],
                            start=(k == 0),
                            stop=(k == KH - 1),
                        )
                # chunked epilogue: sigmoid_i via tanh, xi = (1+t)*bxh,
                # scan with carry, gh = ge*h  (short tail after last matmul)
                ti = tip.tile([128, T], bf16, tag="tip")
                xi = xip.tile([128, T], f32, tag="xip")
                for cq in range(NCQ):
                    s = cq * 512
                    sl = slice(s, s + 512)
                    nc.scalar.activation(ti[:, sl], psi[:, sl], AF.Tanh,
                                         bias=gbh[:, NP + p:NP + p + 1],
                                         scale=0.5)
                    nc.vector.scalar_tensor_tensor(
                        xi[:, sl], ti[:, sl], 1.0, bsc[:, sl],
                        ALU.add, ALU.mult)
                    nc.vector.tensor_tensor_scan(
                        xi[:, sl], alpha[:, sl], xi[:, sl],
                        0.0 if cq == 0 else xi[:, s - 1:s],
                        ALU.mult, ALU.add)
                    nc.gpsimd.tensor_mul(ge[p][:, sl], ge[p][:, sl],
                                         xi[:, sl])

        # ---------------- Phase C: mm3 ----------------
        with (
            tc.tile_pool(name="outp", bufs=2) as outp,
            tc.tile_pool(name="psC", bufs=2, space="PSUM") as psc,
        ):
            for m in range(MD):
                ot = outp.tile([128, T], bf16, tag="outp")
                for h in range(2):
                    hb = h * (T // 2)
                    ps = psc.tile([128, T // 2], f32)
                    for k in range(NP):
                        for cq2 in range(2):
                            cs = cq2 * 512
                            nc.tensor.matmul(
                                ps[:, cs:cs + 512],
                                w3[k][:, m * 128:(m + 1) * 128],
                                ge[k][:, hb + cs:hb + cs + 512],
                                start=(k == 0),
                                stop=(k == NP - 1),
                            )
                    for cq2 in range(2):
                        cs = cq2 * 512
                        sl = slice(hb + cs, hb + cs + 512)
                        if cq2 == 0:
                            nc.scalar.copy(ot[:, sl], ps[:, cs:cs + 512])
                        else:
                            nc.vector.tensor_copy(ot[:, sl],
                                                  ps[:, cs:cs + 512])
                    nc.sync.dma_start(
                        out_d[m * 128:(m + 1) * 128, hb:hb + T // 2],
                        ot[:, hb:hb + T // 2])
        psbi.release()
        w2p.release()
        xhp.release()
        w3p.release()
        gep.release()
        consts.release()

    nc.compile()
    return nc


def make_in_maps(x, input_w, conv_w, conv_b, gates_w, gates_b, forget_base,
                 output_w, D, HID, HS, T, num_cores):
    KD, KH, NP = D // 128, HID // 128, HS // 128
    N = x.shape[0]
    np_bf16 = mybir.dt.np(bf16)
    in_maps = []
    for core in range(num_cores):
        n, c = core // (num_cores // N), core % (num_cores // N)
        own = np.arange(c * HS, (c + 1) * HS)
        other = np.concatenate(
            [np.arange(0, c * HS), np.arange((c + 1) * HS, HID)])
        perm = np.concatenate([own, other])

        xt = np.ascontiguousarray(
            x[n].T.reshape(KD, 128, T).transpose(1, 0, 2)).astype(np_bf16)

        w1sel = np.concatenate([input_w[HID:2 * HID][perm], input_w[own]], 0)
        w1T = w1sel.T  # [D, HID+HS]
        MH = KH + NP
        w1 = np.stack([
            np.ascontiguousarray(
                w1T[:, m * 128:(m + 1) * 128].reshape(KD, 128, 128)
                .transpose(1, 0, 2)).reshape(128, KD * 128)
            for m in range(MH)
        ]).astype(np_bf16)

        cw = np.ascontiguousarray(
            conv_w[perm, 0, :].reshape(KH, 128, 4).transpose(1, 0, 2)
        ).astype(np.float32)
        cb = np.ascontiguousarray(
            conv_b[perm].reshape(KH, 128).T).astype(np.float32)

        w2sel = np.concatenate([gates_w[own], gates_w[HID + own]], 0)
        w2T = w2sel.T[perm]  # [HID(perm order), 2*HS]
        w2 = np.stack([
            np.ascontiguousarray(
                w2T[:, g * 128:(g + 1) * 128].reshape(KH, 128, 128)
                .transpose(1, 0, 2)).reshape(128, KH * 128)
            for g in range(2 * NP)
        ]).astype(np_bf16)

        gbsel = 0.5 * np.concatenate([gates_b[own], gates_b[HID + own]])
        gbt = np.ascontiguousarray(gbsel.reshape(2 * NP, 128).T).astype(np.float32)

        pcv = (-4.0 * np.log1p(np.exp(forget_base.astype(np.float64))))
        pct = np.ascontiguousarray(
            pcv[own].reshape(NP, 128).T).astype(np.float32)

        w3 = np.ascontiguousarray(
            (output_w[:, own].T).reshape(NP, 128, D)).astype(np.float32)

        in_maps.append({
            "xt": xt, "w1": w1, "cw": cw, "cb": cb, "w2": w2,
            "gbh": gbt, "pch": pct, "w3": w3,
        })
    return in_maps


_CACHE = {}
TRACE = False
LAST_RES = None


def _get_nc(D, HID, HS, T, num_cores):
    key = (D, HID, HS, T, num_cores)
    if key not in _CACHE:
        _CACHE[key] = build_nc(D, HID, HS, T, num_cores)
    return _CACHE[key]


def run_hawk(x, input_w, conv_w, conv_b, gates_w, gates_b, forget_base,
             output_w, num_cores=8):
    N, T, D = x.shape
    HID = input_w.shape[0] // 2
    HS = HID // (num_cores // N)
    nc = _get_nc(D, HID, HS, T, num_cores)
    in_maps = make_in_maps(x, input_w, conv_w, conv_b, gates_w, gates_b,
                           forget_base, output_w, D, HID, HS, T, num_cores)
    global LAST_RES
    res = run_bass_kernel_spmd(nc, in_maps, core_ids=list(range(num_cores)),
                               trace=TRACE)
    LAST_RES = res
    shards_per_n = num_cores // N
    out = np.stack([
        sum(res.results[n * shards_per_n + c]["o"].astype(np.float32)
            for c in range(shards_per_n)).T
        for n in range(N)
    ])
    return np.ascontiguousarray(out.astype(np.float32))


def kernel(x, input_w, conv_w, conv_b, gates_w, gates_b, forget_base,
           output_w):
    return run_hawk(
        np.asarray(x, dtype=np.float32),
        np.asarray(input_w, dtype=np.float32),
        np.asarray(conv_w, dtype=np.float32),
        np.asarray(conv_b, dtype=np.float32),
        np.asarray(gates_w, dtype=np.float32),
        np.asarray(gates_b, dtype=np.float32),
        np.asarray(forget_base, dtype=np.float32),
        np.asarray(output_w, dtype=np.float32),
    )


# revision 46
# speedup vs baseline: 1.0879x; 1.0626x over previous
"""Hawk (RG-LRU) Trainium2 kernel.

Full-input contract: kernel(**inputs) takes the unsharded inputs from
setup_inputs() and returns the full [N, T, DIM] output.

Sharding (8 cores): core = 2n + c -> batch n in 0..3, channel-half c in {0,1}
(768 of the 1536 hidden channels). No cross-core communication: each core
computes the FULL conv'd xh (needed as the contraction input of the gates
matmul) but only its channel-half of the gates / recurrence / output
projection. The two per-n partial outputs are summed on the host.

Per-core pipeline, channels-on-partitions / time-on-free, bf16 matmuls:
  A) xt resident in SBUF (bf16, streamed in 512-col chunks), mm1 with
     w1 streamed once (no re-loads), xh tiles in bf16, causal depthwise
     conv as 4 shifted multiply-adds (DVE/Pool split), gate tiles via
     direct Gelu on ACT.
  B) per channel-tile p: forget/input matmuls (K=1536) into PSUM,
     sigmoid/exp via tanh identities (single ACT table + sqrt),
     alpha = exp(pc*sig(f)), bxh = 0.5*beta*xh, xi = (1+tanh_i)*bxh,
     recurrence via chunked tensor_tensor_scan with carry, gh = ge*h.
  C) out = W3 @ gh (K=768), DMA straight from PSUM.
"""

import numpy as np

import concourse.bacc as bacc
import concourse.mybir as mybir
import concourse.tile as tile
from concourse.bass_utils import run_bass_kernel_spmd

f32 = mybir.dt.float32
f32r = mybir.dt.float32r
bf16 = mybir.dt.bfloat16
AF = mybir.ActivationFunctionType
ALU = mybir.AluOpType


def build_nc(D, HID, HS, T, num_cores=8):
    """Build + compile the per-core SPMD program. All cores run this same
    program on different data."""
    KD, KH, NP = D // 128, HID // 128, HS // 128
    MH = KH + NP          # mm1 output tiles: KH xh tiles then NP gate tiles
    MD = D // 128
    NCQ = T // 512        # 512-col chunks

    nc = bacc.Bacc("TRN2", target_bir_lowering=False, debug=False,
                   num_devices=num_cores)

    xt_d = nc.dram_tensor("xt", [128, KD, T], bf16, kind="ExternalInput")
    w1_d = nc.dram_tensor("w1", [MH, 128, KD * 128], bf16, kind="ExternalInput")
    cw_d = nc.dram_tensor("cw", [128, KH, 4], f32, kind="ExternalInput")
    cb_d = nc.dram_tensor("cb", [128, KH], f32, kind="ExternalInput")
    w2_d = nc.dram_tensor("w2", [2 * NP, 128, KH * 128], bf16, kind="ExternalInput")
    gbh_d = nc.dram_tensor("gbh", [128, 2 * NP], f32, kind="ExternalInput")
    pch_d = nc.dram_tensor("pch", [128, NP], f32, kind="ExternalInput")
    w3_d = nc.dram_tensor("w3", [NP, 128, D], f32r, kind="ExternalInput")
    out_d = nc.dram_tensor("o", [D, T], bf16, kind="ExternalOutput")

    POOL_CONV = ()   # TensorScalarPtr is DVE-only on real HW (no Pool conv)

    with tile.TileContext(nc) as tc:
        # pool alloc order is LIFO-release: consts/ge/w3 live to the end,
        # xh dies after phase B, xt after phase A
        consts = tc.alloc_tile_pool(name="consts", bufs=1)
        gep = tc.alloc_tile_pool(name="ge", bufs=NP)
        w3p = tc.alloc_tile_pool(name="w3", bufs=NP)
        xhp = tc.alloc_tile_pool(name="xh", bufs=KH)
        # w2 sits below xt on the alloc stack so its tiles don't alias xt's
        # space -> phase B's first weight DMAs don't wait for xt to die
        w2p = tc.alloc_tile_pool(name="w2", bufs=4)
        xtp = tc.alloc_tile_pool(name="xt", bufs=1)

        # x resident for all of phase A, streamed chunk-major so the first
        # matmul only waits on the first 512-col chunk. DMA issue order
        # matters: the single DMA pipe drains in order, so get the first
        # chunk + first w1 tiles out before the bulk of x.
        xt = xtp.tile([128, KD, T], bf16, tag="xt")
        w1p = tc.alloc_tile_pool(name="w1", bufs=3)
        w1head = []

        nc.sync.dma_start(xt[:, :, 0:256], xt_d[:, :, 0:256])
        w1m0 = w1p.tile([128, KD, 128], bf16, tag="w1")
        nc.sync.dma_start(w1m0[:], w1_d[0].rearrange("p (k f) -> p k f", k=KD))
        w1head.append(w1m0)
        nc.sync.dma_start(xt[:, :, 256:512], xt_d[:, :, 256:512])
        for m in range(1, 3):
            w1m = w1p.tile([128, KD, 128], bf16, tag="w1")
            nc.sync.dma_start(
                w1m[:], w1_d[m].rearrange("p (k f) -> p k f", k=KD))
            w1head.append(w1m)

        for cq in range(1, NCQ):
            s = cq * 512
            nc.sync.dma_start(xt[:, :, s:s + 512], xt_d[:, :, s:s + 512])

        cw = consts.tile([128, KH, 4], f32, tag="cw")
        nc.sync.dma_start(cw[:], cw_d[:])
        cb = consts.tile([128, KH], f32, tag="cb")
        nc.sync.dma_start(cb[:], cb_d[:])
        gbh = consts.tile([128, 2 * NP], f32, tag="gbh")
        nc.sync.dma_start(gbh[:], gbh_d[:])
        pch = consts.tile([128, NP], f32, tag="pch")
        nc.sync.dma_start(pch[:], pch_d[:])
        qrt = consts.tile([128, 1], f32, tag="qrt")
        nc.gpsimd.memset(qrt[:], 0.25)

        # gelu(gate) tiles, resident through phase C (become gh in phase B)
        ge = [gep.tile([128, T], f32r, tag="ge", name=f"ge{g}")
              for g in range(NP)]

        xh = [xhp.tile([128, T + 4], bf16, tag="xh", name=f"xh{m}")
              for m in range(KH)]
        for m in range(KH):
            nc.gpsimd.memset(xh[m][:, 0:4].bitcast(f32), 0.0)

        # ---------------- Phase A: mm1 + conv + gelu(gate) ----------------
        with (
            tc.tile_pool(name="accv", bufs=2) as accvp,
            tc.tile_pool(name="accp", bufs=2) as accpp,
            tc.tile_pool(name="psA", bufs=4, space="PSUM") as psa,
        ):
            for m in range(MH):
                if m < 3:
                    w1m = w1head[m]
                else:
                    w1m = w1p.tile([128, KD, 128], bf16, tag="w1")
                    nc.sync.dma_start(
                        w1m[:], w1_d[m].rearrange("p (k f) -> p k f", k=KD))
                # m == 0 runs its first 512 cols as 2x256 so the very first
                # matmul only waits on the leading 256-col x DMA
                if m == 0:
                    chunks = [(0, 256), (256, 256)] + [
                        (s, 512) for s in range(512, T, 512)]
                else:
                    chunks = [(s, 512) for s in range(0, T, 512)]
                for s, CA in chunks:
                    ps = psa.tile([128, 512], f32)
                    for k in range(KD):
                        nc.tensor.matmul(
                            ps[:, 0:CA],
                            w1m[:, k, :],
                            xt[:, k, s:s + CA],
                            start=(k == 0),
                            stop=(k == KD - 1),
                        )
                    if m < KH:
                        nc.scalar.copy(xh[m][:, 4 + s:4 + s + CA],
                                       ps[:, 0:CA])
                    else:
                        nc.scalar.activation(ge[m - KH][:, s:s + CA],
                                             ps[:, 0:CA], AF.Gelu)
                if m < KH:
                    # causal depthwise conv, 4 shifted multiply-adds.
                    # taps 0-2 read raw (not yet convolved) columns via the
                    # 4-zero left pad; the final tap overwrites in place.
                    eng = nc.gpsimd if m in POOL_CONV else nc.vector
                    accp_ = accpp if m in POOL_CONV else accvp
                    acc = accp_.tile([128, T], bf16, tag="acc")
                    eng.tensor_scalar(
                        acc[:], xh[m][:, 1:1 + T],
                        cw[:, m, 0:1], cb[:, m:m + 1],
                        ALU.mult, ALU.add)
                    for tap in (1, 2):
                        eng.scalar_tensor_tensor(
                            acc[:], xh[m][:, 1 + tap:1 + tap + T],
                            cw[:, m, tap:tap + 1],
                            acc[:], ALU.mult, ALU.add)
                    eng.scalar_tensor_tensor(
                        xh[m][:, 4:4 + T], xh[m][:, 4:4 + T],
                        cw[:, m, 3:4], acc[:], ALU.mult, ALU.add)
        w1p.release()
        xtp.release()

        # ---------------- Phase B: mm2 + gates + scan + gh ----------------
        # psBi outlives phase B: if it were released with the other B pools,
        # phase C's PSUM tiles would alias it and wait on its last reader
        # (the final input-gate tanh) before C's first matmul could start.
        psbi = tc.alloc_tile_pool(name="psBi", bufs=1, space="PSUM")
        with (
            tc.tile_pool(name="alp", bufs=2) as alp,
            tc.tile_pool(name="bsc", bufs=2) as bscp,
            tc.tile_pool(name="tip", bufs=2) as tip,
            tc.tile_pool(name="xip", bufs=2) as xip,
            tc.tile_pool(name="psBf", bufs=1, space="PSUM") as psbf,
        ):
            # first gate-weight pair ahead of the w3 prefetch: the DMA pipe
            # drains in order and phase B's first matmul needs w2[0]
            w2head = []
            for g in (0, NP):
                w2g = w2p.tile([128, KH, 128], bf16, tag="w2")
                nc.sync.dma_start(
                    w2g[:], w2_d[g].rearrange("p (k f) -> p k f", k=KH))
                w2head.append(w2g)
            # w3 prefetch (used in phase C; DMA overlaps phase B compute)
            w3 = []
            for k in range(NP):
                w3k = w3p.tile([128, D], f32r, tag="w3", name=f"w3_{k}")
                nc.sync.dma_start(w3k[:], w3_d[k])
                w3.append(w3k)

            for p in range(NP):
                # forget gate matmul
                if p == 0:
                    w2f = w2head[0]
                else:
                    w2f = w2p.tile([128, KH, 128], bf16, tag="w2")
                    nc.sync.dma_start(
                        w2f[:], w2_d[p].rearrange("p (k f) -> p k f", k=KH))
                psf = psbf.tile([128, T], f32, tag="psBf")
                for k in range(KH):
                    for h in range(NCQ):
                        hs = h * 512
                        nc.tensor.matmul(
                            psf[:, hs:hs + 512],
                            w2f[:, k, :],
                            xh[k][:, 4 + hs:4 + hs + 512],
                            start=(k == 0),
                            stop=(k == KH - 1),
                        )
                # alpha = exp(pc * sigmoid(f)) via tanh identity:
                #   sigmoid(f) = 0.5 + 0.5*tanh(0.5 f);  pch = 0.5*pc
                alpha = alp.tile([128, T], f32, tag="alp")
                nc.scalar.activation(alpha[:], psf[:], AF.Tanh,
                                     bias=gbh[:, p:p + 1], scale=0.5)
                nc.scalar.activation(alpha[:], alpha[:], AF.Exp,
                                     bias=pch[:, p:p + 1],
                                     scale=pch[:, p:p + 1])
                # bxh = 0.5*beta*xh, with 0.5*beta = sqrt(0.25 - 0.25 a^2)
                bsc = bscp.tile([128, T], f32, tag="bsc")
                nc.vector.tensor_mul(bsc[:], alpha[:], alpha[:])
                nc.scalar.activation(bsc[:], bsc[:], AF.Sqrt,
                                     bias=qrt[:, 0:1], scale=-0.25)
                nc.vector.tensor_mul(bsc[:], bsc[:], xh[p][:, 4:4 + T])
                # input gate matmul
                if p == 0:
                    w2i = w2head[1]
                else:
                    w2i = w2p.tile([128, KH, 128], bf16, tag="w2")
                    nc.sync.dma_start(
                        w2i[:], w2_d[NP + p].rearrange("p (k f) -> p k f", k=KH))
                psi = psbi.tile([128, T], f32, tag="psBi")
                for k in range(KH):
                    for h in range(NCQ):
                        hs = h * 512
                        nc.tensor.matmul(
                            psi[:, hs:hs + 512],
                            w2i[:, k, :],
                            xh[k][:, 4 + hs:4 + hs + 512],
                            start=(k == 0),
                            stop=(k == KH - 1),
                        )
                # chunked epilogue: sigmoid_i via tanh, xi = (1+t)*bxh,
                # scan with carry, gh = ge*h  (short tail after last matmul)
                ti = tip.tile([128, T], bf16, tag="tip")
                xi = xip.tile([128, T], f32, tag="xip")
                for cq in range(NCQ):
                    s = cq * 512
                    sl = slice(s, s + 512)
                    nc.scalar.activation(ti[:, sl], psi[:, sl], AF.Tanh,
                                         bias=gbh[:, NP + p:NP + p + 1],
                                         scale=0.5)
                    nc.vector.scalar_tensor_tensor(
                        xi[:, sl], ti[:, sl], 1.0, bsc[:, sl],
                        ALU.add, ALU.mult)
                    nc.vector.tensor_tensor_scan(
                        xi[:, sl], alpha[:, sl], xi[:, sl],
                        0.0 if cq == 0 else xi[:, s - 1:s],
                        ALU.mult, ALU.add)
                    nc.gpsimd.tensor_mul(ge[p][:, sl], ge[p][:, sl],
                                         xi[:, sl])

        # ---------------- Phase C: mm3 ----------------
        with (
            tc.tile_pool(name="outp", bufs=2) as outp,
            tc.tile_pool(name="psC", bufs=2, space="PSUM") as psc,
        ):
            for m in range(MD):
                ot = outp.tile([128, T], bf16, tag="outp")
                for h in range(2):
                    hb = h * (T // 2)
                    ps = psc.tile([128, T // 2], f32)
                    for k in range(NP):
                        for cq2 in range(2):
                            cs = cq2 * 512
                            nc.tensor.matmul(
                                ps[:, cs:cs + 512],
                                w3[k][:, m * 128:(m + 1) * 128],
                                ge[k][:, hb + cs:hb + cs + 512],
                                start=(k == 0),
                                stop=(k == NP - 1),
                            )
                    for cq2 in range(2):
                        cs = cq2 * 512
                        sl = slice(hb + cs, hb + cs + 512)
                        if cq2 == 0:
                            nc.scalar.copy(ot[:, sl], ps[:, cs:cs + 512])
                        else:
                            nc.vector.tensor_copy(ot[:, sl],
                                                  ps[:, cs:cs + 512])
                    nc.sync.dma_start(
                        out_d[m * 128:(m + 1) * 128, hb:hb + T // 2],
                        ot[:, hb:hb + T // 2])
        psbi.release()
        w2p.release()
        xhp.release()
        w3p.release()
        gep.release()
        consts.release()

    nc.compile()
    return nc


def make_in_maps(x, input_w, conv_w, conv_b, gates_w, gates_b, forget_base,
                 output_w, D, HID, HS, T, num_cores):
    KD, KH, NP = D // 128, HID // 128, HS // 128
    N = x.shape[0]
    np_bf16 = mybir.dt.np(bf16)
    in_maps = []
    for core in range(num_cores):
        n, c = core // (num_cores // N), core % (num_cores // N)
        own = np.arange(c * HS, (c + 1) * HS)
        other = np.concatenate(
            [np.arange(0, c * HS), np.arange((c + 1) * HS, HID)])
        perm = np.concatenate([own, other])

        xt = np.ascontiguousarray(
            x[n].T.reshape(KD, 128, T).transpose(1, 0, 2)).astype(np_bf16)

        w1sel = np.concatenate([input_w[HID:2 * HID][perm], input_w[own]], 0)
        w1T = w1sel.T  # [D, HID+HS]
        MH = KH + NP
        w1 = np.stack([
            np.ascontiguousarray(
                w1T[:, m * 128:(m + 1) * 128].reshape(KD, 128, 128)
                .transpose(1, 0, 2)).reshape(128, KD * 128)
            for m in range(MH)
        ]).astype(np_bf16)

        cw = np.ascontiguousarray(
            conv_w[perm, 0, :].reshape(KH, 128, 4).transpose(1, 0, 2)
        ).astype(np.float32)
        cb = np.ascontiguousarray(
            conv_b[perm].reshape(KH, 128).T).astype(np.float32)

        w2sel = np.concatenate([gates_w[own], gates_w[HID + own]], 0)
        w2T = w2sel.T[perm]  # [HID(perm order), 2*HS]
        w2 = np.stack([
            np.ascontiguousarray(
                w2T[:, g * 128:(g + 1) * 128].reshape(KH, 128, 128)
                .transpose(1, 0, 2)).reshape(128, KH * 128)
            for g in range(2 * NP)
        ]).astype(np_bf16)

        gbsel = 0.5 * np.concatenate([gates_b[own], gates_b[HID + own]])
        gbt = np.ascontiguousarray(gbsel.reshape(2 * NP, 128).T).astype(np.float32)

        pcv = (-4.0 * np.log1p(np.exp(forget_base.astype(np.float64))))
        pct = np.ascontiguousarray(
            pcv[own].reshape(NP, 128).T).astype(np.float32)

        w3 = np.ascontiguousarray(
            (output_w[:, own].T).reshape(NP, 128, D)).astype(np.float32)

        in_maps.append({
            "xt": xt, "w1": w1, "cw": cw, "cb": cb, "w2": w2,
            "gbh": gbt, "pch": pct, "w3": w3,
        })
    return in_maps


_CACHE = {}
TRACE = False
LAST_RES = None


def _get_nc(D, HID, HS, T, num_cores):
    key = (D, HID, HS, T, num_cores)
    if key not in _CACHE:
        _CACHE[key] = build_nc(D, HID, HS, T, num_cores)
    return _CACHE[key]


def run_hawk(x, input_w, conv_w, conv_b, gates_w, gates_b, forget_base,
             output_w, num_cores=8):
    N, T, D = x.shape
    HID = input_w.shape[0] // 2
    HS = HID // (num_cores // N)
    nc = _get_nc(D, HID, HS, T, num_cores)
    in_maps = make_in_maps(x, input_w, conv_w, conv_b, gates_w, gates_b,
                           forget_base, output_w, D, HID, HS, T, num_cores)
    global LAST_RES
    res = run_bass_kernel_spmd(nc, in_maps, core_ids=list(range(num_cores)),
                               trace=TRACE)
    LAST_RES = res
    shards_per_n = num_cores // N
    out = np.stack([
        sum(res.results[n * shards_per_n + c]["o"].astype(np.float32)
            for c in range(shards_per_n)).T
        for n in range(N)
    ])
    return np.ascontiguousarray(out.astype(np.float32))


def kernel(x, input_w, conv_w, conv_b, gates_w, gates_b, forget_base,
           output_w):
    return run_hawk(
        np.asarray(x, dtype=np.float32),
        np.asarray(input_w, dtype=np.float32),
        np.asarray(conv_w, dtype=np.float32),
        np.asarray(conv_b, dtype=np.float32),
        np.asarray(gates_w, dtype=np.float32),
        np.asarray(gates_b, dtype=np.float32),
        np.asarray(forget_base, dtype=np.float32),
        np.asarray(output_w, dtype=np.float32),
    )
